# revision 3
# baseline (speedup 1.0000x reference)
"""nn_BackgroundLoss segment-reduce kernel for 8 Trainium2 NeuronCores.

Contract: kernel(**inputs) takes the FULL unsharded inputs (w, beta, x, y,
particle_id as numpy arrays; only beta/particle_id feed the math) and
returns the full output (a float32 scalar), running the reduction on the 8
NeuronCores via a Bass/Tile SPMD kernel.

Algorithm (exact, bucketed segment max):
  The loss needs seg_max[p] = max beta over hits of particle p (P=50000
  segments), the count of non-empty segments, and the noise (pid==0)
  mean.  pid space is padded to 50176 = 8 cores x 6272 pids and range-
  partitioned over cores; each core's 6272 pids map to (partition,
  column) buckets of a [128, ncols] float16 tile whose beta values the
  host scatters in bucket order (pad = -65504), a pure layout
  permutation; fp16 only perturbs each seg_max by <=2^-11 relative,
  ~1e-6 on the loss.  To cut padding, each core's pids are sorted by
  hit count and dealt into 5 column groups of decreasing uniform width
  W_s (the group's max count, shared across cores so the SPMD program
  is identical); total columns ~ 6272*mean_count/128 instead of
  49*max_count.  On device each group is DMA'd as one slice and folded
  W->W/2->W/4 with two cheap tensor_tensor(max) passes plus a mini
  tensor_reduce(max) as it lands, overlapped with the remaining
  transfers.  Per-bucket maxes collapse to a [128,4] partial =
  [sum relu(max), n_valid, noise_sum, noise_cnt] (empty buckets keep
  the pad and fail the >= -0.5 validity test; sum(1-max) over valid
  buckets = n_valid - sum relu(max) since real maxes are >= 0).  Noise
  hits (pid 0) stay out of the main tile and ride a separate [1, W0]
  float32 row given only to core 0, reduced with relu / is_ge.  The
  host gathers the per-core [128,4] partials, sums them, and forms the
  scalar (attract/n_valid + SB*noise_mean).

Sharding: pids (not hits) are range-partitioned over cores, so each
core's ~250k hits are private to it and nothing but the [128,4]
partial tile leaves a core.
"""
import sys

if '/opt/trn_rl_repo' not in sys.path:
    sys.path.insert(0, '/opt/trn_rl_repo')

import numpy as np
from concourse import bacc, tile, mybir
from concourse.bass_utils import run_bass_kernel_spmd

F32 = mybir.dt.float32
F16 = mybir.dt.float16
Alu = mybir.AluOpType

SB = 0.1
NUM_PIDS = 50_000
NH = 49              # pid columns per partition
N_CORES = 8
PPC = 128 * NH       # 6272 pids per core; 8*6272 = 50176 >= 50000
PAD16 = np.float16(-65504.0)
PAD32 = np.float32(-1e30)
SLICES = (10, 10, 10, 10, 9)  # h-groups (count-rank bands) per DMA slice

_cache: dict = {}


def _build(n_cores: int, Ws: tuple):
    nc = bacc.Bacc("TRN2", target_bir_lowering=False, debug=False,
                   num_devices=n_cores)
    ncols = sum(hg * w for hg, w in zip(SLICES, Ws))
    beta_d = nc.dram_tensor("beta", [128, ncols], F16,
                            kind="ExternalInput").ap()
    nz_d = nc.dram_tensor("nz", [1, Ws[0]], F32, kind="ExternalInput").ap()
    y_d = nc.dram_tensor("y", [128, 4], F32, kind="ExternalOutput").ap()

    with tile.TileContext(nc) as tc:
        with (
            tc.tile_pool(name="const", bufs=1) as constp,
            tc.tile_pool(name="bulk", bufs=1) as bulkp,
            tc.tile_pool(name="fin", bufs=1) as finp,
        ):
            bts = []
            off = 0    # h offset
            coff = 0   # column offset in beta_d
            for g, (hg, w) in enumerate(zip(SLICES, Ws)):
                bt = bulkp.tile([128, hg * w], F16, tag=f"bt{g}")
                eng = nc.sync if g % 2 == 0 else nc.scalar
                eng.dma_start(out=bt[:], in_=beta_d[:, coff:coff + hg * w])
                bts.append((bt, off, hg, w))
                off += hg
                coff += hg * w

            # tiny const behind sync's bulk slices; consumed only in the
            # late epilogue, so the queueing delay is immaterial
            nz = constp.tile([1, Ws[0]], F32, tag="nz")
            nc.sync.dma_start(out=nz[:], in_=nz_d[:])

            # per-slice W -> W/4 max-tree + mini-reduce as the slices land
            tmax = finp.tile([128, NH], F16, tag="tmax")
            for bt, off, hg, w in bts:
                w2, w4 = w // 2, w // 4
                b3 = bt[:].rearrange("p (h w) -> p h w", w=w)
                t1 = bulkp.tile([128, hg * w2], F16, tag=f"t1_{off}")
                t13 = t1[:].rearrange("p (h w) -> p h w", w=w2)
                nc.vector.tensor_tensor(t13, b3[:, :, 0:w2], b3[:, :, w2:w],
                                        Alu.max)
                q = bulkp.tile([128, hg * w4], F16, tag=f"q_{off}")
                q3 = q[:].rearrange("p (h w) -> p h w", w=w4)
                nc.vector.tensor_tensor(q3, t13[:, :, 0:w4], t13[:, :, w4:w2],
                                        Alu.max)
                nc.vector.tensor_reduce(tmax[:, off:off + hg], q3,
                                        mybir.AxisListType.X, Alu.max)

            # S = [sum relu(max), n_valid, noise_sum, noise_cnt]
            S = finp.tile([128, 4], F32, tag="S")
            vr = finp.tile([128, 2 * NH], F32, tag="vr")
            nc.vector.memset(S[:], 0.0)
            nc.vector.tensor_scalar_max(vr[:, 0:NH], tmax[:], 0.0)
            nc.vector.tensor_scalar(vr[:, NH:2 * NH], tmax[:], -0.5, None,
                                    Alu.is_ge)
            nc.vector.tensor_reduce(S[:, 0:2],
                                    vr[:].rearrange("p (h w) -> p h w", w=NH),
                                    mybir.AxisListType.X, Alu.add)
            # noise partials (partition 0 only): relu kills pads, is_ge counts
            nrelu = finp.tile([1, Ws[0]], F32, tag="nrelu")
            nmask = finp.tile([1, Ws[0]], F32, tag="nmask")
            nc.vector.tensor_scalar_max(nrelu[:], nz[:], 0.0)
            nc.vector.tensor_scalar(nmask[:], nz[:], -0.5, None, Alu.is_ge)
            nc.vector.tensor_reduce(S[0:1, 2:3], nrelu[:],
                                    mybir.AxisListType.X, Alu.add)
            nc.vector.tensor_reduce(S[0:1, 3:4], nmask[:],
                                    mybir.AxisListType.X, Alu.add)

            nc.sync.dma_start(out=y_d[:], in_=S[:])

    nc.compile()
    return nc


def _shard(beta: np.ndarray, pid: np.ndarray):
    """Scatter each hit's beta into its pid bucket's row segment.

    Per core, pids are ranked by hit count (desc) and dealt into
    count-rank bands of 128 pids per h; band groups share a uniform
    width W_s = the band group's max count across all cores.  Returns
    per-core [128, ncols] float16 maps (pad -65504), the [1, W_0]
    float32 noise row for core 0, and the width tuple Ws.  pid-0 hits
    go only to the noise row, so their bucket stays empty/invalid.
    """
    n = beta.shape[0]
    order = np.argsort(pid, kind="stable")
    pid_s = pid[order].astype(np.int64)
    counts = np.bincount(pid_s, minlength=PPC * N_CORES)
    starts = np.concatenate([[0], np.cumsum(counts)[:-1]])
    rank = np.arange(n, dtype=np.int64) - starts[pid_s]
    beta_s = beta[order].astype(np.float16)

    cnts = counts.reshape(N_CORES, PPC).copy()
    cnts[0, 0] = 0  # noise pids ride the nz row, keep its bucket empty
    csort = -np.sort(-cnts, axis=1)  # per-core counts, descending
    bounds = np.concatenate([[0], np.cumsum([hg * 128 for hg in SLICES])])
    Ws = tuple(int((int(csort[:, bounds[s]].max()) + 3) // 4 * 4)
               for s in range(len(SLICES)))

    # per-core slot maps: local pid -> (row, column base)
    c = pid_s // PPC
    lp = pid_s - c * PPC
    maps = []
    nz = np.full((1, Ws[0]), PAD32, np.float32)
    n0 = int(counts[0])
    nz[0, :n0] = beta[order][:n0]  # sorted order puts pid 0 first
    ncols = sum(hg * w for hg, w in zip(SLICES, Ws))
    for cc in range(N_CORES):
        ordd = np.argsort(-cnts[cc], kind="stable")
        colbase = np.empty(PPC, np.int64)
        rowidx = np.empty(PPC, np.int64)
        coff = 0
        for s, hg in enumerate(SLICES):
            sl = ordd[bounds[s]:bounds[s + 1]]
            idx = np.arange(hg * 128)
            colbase[sl] = coff + (idx // 128) * Ws[s]
            rowidx[sl] = idx % 128
            coff += hg * Ws[s]
        sel = c == cc
        if cc == 0:
            sel = sel & (pid_s > 0)
        buf = np.full((128, ncols), PAD16, np.float16)
        lps = lp[sel]
        buf[rowidx[lps], colbase[lps] + rank[sel]] = beta_s[sel]
        maps.append(buf)
    return maps, nz, Ws


def _finalize(partials):
    """Gather: sum the per-core [128,4] partials, form the scalar."""
    G = np.asarray(partials, dtype=np.float32).sum(axis=(0, 1),
                                                   dtype=np.float32)
    n_valid = max(G[1], np.float32(1.0))
    attract = (G[1] - G[0]) / n_valid
    noise = G[2] / max(G[3], np.float32(1.0))
    return np.float32(attract + np.float32(SB) * noise)


def _in_maps(maps, nz, Ws):
    pad_nz = np.full((1, Ws[0]), PAD32, np.float32)
    return [
        {"beta": m, "nz": nz if c == 0 else pad_nz}
        for c, m in enumerate(maps)
    ]


def kernel(w, beta, x, y, particle_id):
    beta = np.ascontiguousarray(np.asarray(beta, dtype=np.float32))
    pid = np.ascontiguousarray(np.asarray(particle_id, dtype=np.int32))

    maps, nz, Ws = _shard(beta, pid)
    key = (N_CORES, Ws)
    if key not in _cache:
        _cache[key] = _build(N_CORES, Ws)
    nc = _cache[key]

    in_maps = _in_maps(maps, nz, Ws)
    res = run_bass_kernel_spmd(nc, in_maps, list(range(N_CORES))).results
    return np.asarray(_finalize([r["y"] for r in res]), dtype=np.float32)
